# revision 39
# baseline (speedup 1.0000x reference)
"""Causal self-attention with RoPE (B=2, T=2048, C=2048, H=16, D=128) on 8 TRN2 cores.

Sharding: tensor-parallel over heads (2 heads per core).
  - column-parallel fused QKV projection (each core computes q,k,v for its 2 heads)
  - RoPE + causal flash-style attention per (batch, head) on-core
  - AllToAll to regroup the attention output from head-sharded to token-sharded
  - token-parallel output projection (each core produces 512 token rows of y)

v6: fine-grained phase interleaving so the PE array never idles (p-state):
  A: QKV b0 blocks with attn b0 (qt=0,1) units woven between blocks
     (the attn PE work covers the x-stream block-boundary deficit)
  B: attn b0 (qt=2,3) + attn b1 (qt=0,1) + QKV b1 blocks; b0 spill +
     AllToAll fire mid-phase, readback at phase end
  C: attn b1 (qt=2,3) + one proj hb0 unit (collective b0 long done)
  D: proj hb0 rest (hides the b1 AllToAll) then proj hb1
x is streamed in 4-chunk grouped DMAs (fewer queue issues); a2a spill is
1 DMA per (batch, head), readback 1 DMA per batch.

Queues: sync = x-stream, spills, ya readback, y out; scalar = weights + v
drains; gpsimd = rope tables + rotates + collectives; vector = qk drains,
rope muls, attention softmax scale.

Layouts (per core):
  x3      (8, 2048, 512)      bf16   x^T in 512-token blocks, replicated
  wqk     (16, 128, 512)      bf16   [c-chunk, c, q_h0|q_h1|k_h0|k_h1]
  wv      (16, 128, 256)      bf16   [c-chunk, c, v_h0|v_h1]
  wproj   (16, 128, 2048)     bf16   w_proj.T chunked, replicated
  cosT    (128, 2048)         bf16   RoPE cos, (D, T)
  sinTs   (128, 2048)         bf16   RoPE sin, (D, T), rows 0:64 negated
  tri     (128, 128)          bf16   lower-tri 0/1 mask (kp <= qf)
  y_out   (512, 2048)         f32    output rows for this core's token slice
"""

import contextlib

import numpy as np
import ml_dtypes

import concourse.bass as bass
import concourse.bacc as bacc
import concourse.mybir as mybir
import concourse.tile as tile
from concourse import masks as cmasks
from concourse.bass_utils import run_bass_kernel_spmd

N_CORES = 8
B, T, C = 2, 2048, 2048
H, D = 16, 128
H_LOC = H // N_CORES          # 2 heads per core
BT = B * T                    # 4096 tokens
TOK_PC = BT // N_CORES        # 512 tokens per core (proj phase)
SCALE = 1.0 / float(np.sqrt(D))
ROPE_BASE = 10000.0

F32 = mybir.dt.float32
BF16 = mybir.dt.bfloat16

N_XB = BT // 512              # 8 x DMA blocks of 512 tokens
N_BB = T // 512               # 4 blocks of 512 per batch
N_CCH = C // 128              # 16 contraction chunks
N_XG = 4                      # x chunks per grouped DMA


def build():
    nc = bacc.Bacc("TRN2", target_bir_lowering=False, debug=False,
                   num_devices=N_CORES)

    x3_d = nc.dram_tensor("x3", [N_XB, C, 512], BF16, kind="ExternalInput").ap()
    wqk_d = nc.dram_tensor("wqk", [N_CCH, 128, 512], BF16, kind="ExternalInput").ap()
    wv_d = nc.dram_tensor("wv", [N_CCH, 128, 256], BF16, kind="ExternalInput").ap()
    wproj_d = nc.dram_tensor("wproj", [N_CCH, 128, C], BF16, kind="ExternalInput").ap()
    cosT_d = nc.dram_tensor("cosT", [128, T], BF16, kind="ExternalInput").ap()
    sinTs_d = nc.dram_tensor("sinTs", [128, T], BF16, kind="ExternalInput").ap()
    tri_d = nc.dram_tensor("tri", [128, 128], BF16, kind="ExternalInput").ap()
    y_d = nc.dram_tensor("y", [TOK_PC, C], F32, kind="ExternalOutput").ap()

    a2a_in = [nc.dram_tensor(f"a2a_in{i}", [N_CORES, 256, 256], BF16).ap()
              for i in range(2)]
    a2a_out = [nc.dram_tensor(f"a2a_out{i}", [N_CORES, 256, 256], BF16).ap()
               for i in range(2)]
    cc_warm_in = nc.dram_tensor("ccw_in", [N_CORES, 1, 64], BF16).ap()
    cc_warm_out = nc.dram_tensor("ccw_out", [N_CORES, 1, 64], BF16).ap()

    tensors = dict(x3_d=x3_d, wqk_d=wqk_d, wv_d=wv_d, wproj_d=wproj_d,
                   cosT_d=cosT_d, sinTs_d=sinTs_d, tri_d=tri_d, y_d=y_d,
                   a2a_in=a2a_in, a2a_out=a2a_out,
                   cc_warm_in=cc_warm_in, cc_warm_out=cc_warm_out)
    with tile.TileContext(nc) as tc:
        _emit(nc, tc, tensors)
    nc.compile()
    return nc


# ---------------- generators (emission units) ----------------

def _gen_qkv(nc, b, x3_d, wqk_sb, wv_sb, cos_sb, sin_sb, qk_sb, v_sb, P):
    """QKV projection + RoPE for batch b; yields after each 512-token block."""
    for blk in range(N_BB):
        xb = b * N_BB + blk
        pos0 = 512 * blk
        xg = []
        for g in range(N_CCH // N_XG):
            xt = P["xp"].tile([128, N_XG, 512], BF16, name="xt", tag="xt")
            nc.sync.dma_start(
                out=xt[:],
                in_=x3_d[xb, g * N_XG * 128:(g + 1) * N_XG * 128, :]
                .rearrange("(q p) c -> p q c", p=128))
            xg.append(xt)

        def xch(c):
            return xg[c // N_XG][:, c % N_XG, :]

        # q/k: j-outer, c-inner, one PSUM bank per j, N=512 streams.
        # RoPE is pipelined per j: each j's rotate-half DMA is issued right
        # after its drain (overlapping later drains), and the multiply pass
        # runs once all rotates are in flight — no block-end rope backlog.
        ropes = []
        for j in range(4):
            # j: 0=q_h0 1=q_h1 2=k_h0 3=k_h1 (wqk col offset 128*j)
            qk_ps = P["qkps"].tile([128, 512], F32, name="qk_ps", tag="qk_ps")
            for c in range(N_CCH):
                nc.tensor.matmul(qk_ps[:],
                                 wqk_sb[:, c, j * 128:(j + 1) * 128],
                                 xch(c), start=(c == 0), stop=(c == N_CCH - 1))
            bh = b * 2 + (j % 2)
            dslc = qk_sb[:, j // 2, bh, pos0:pos0 + 512]
            nc.vector.tensor_copy(out=dslc, in_=qk_ps[:])
            rot = P["rp"].tile([128, 512], BF16, name="rot", tag="rot")
            nc.gpsimd.dma_start(out=rot[0:64, :], in_=dslc[64:128])
            nc.gpsimd.dma_start(out=rot[64:128, :], in_=dslc[0:64])
            ropes.append((dslc, rot))
        for dslc, rot in ropes:
            nc.vector.tensor_mul(out=rot[:], in0=rot[:],
                                 in1=sin_sb[:, pos0:pos0 + 512])
            nc.vector.tensor_mul(out=dslc, in0=dslc,
                                 in1=cos_sb[:, pos0:pos0 + 512])
            nc.vector.tensor_add(out=dslc, in0=dslc, in1=rot[:])
        # v: two 256-token halves, each one PSUM bank with 2 token-chunk groups
        for half in range(2):
            v_ps = P["vps"].tile([128, 2, 256], F32, name="v_ps", tag="v_ps")
            for c in range(N_CCH):
                # PSUM start=True clears has_written BANK-wide, so only the
                # first group in the bank may use it; the second group's
                # first write overwrites via the cleared has_written bit.
                for ts in range(2):
                    nc.tensor.matmul(v_ps[:, ts, :],
                                     xch(c)[:, half * 256 + ts * 128:
                                            half * 256 + (ts + 1) * 128],
                                     wv_sb[:, c, :],
                                     start=(c == 0 and ts == 0),
                                     stop=(c == N_CCH - 1),
                                     skip_group_check=True)
            for ts in range(2):
                kt = 4 * blk + 2 * half + ts
                for h in range(2):
                    nc.scalar.copy(out=v_sb[:, b * 2 + h, kt, 0:128],
                                   in_=v_ps[:, ts, h * 128:(h + 1) * 128])
        yield


def _gen_attention(nc, b, qts, qk_sb, v_sb, yT_sb, tri_sb, ident, P):
    """Flash attention for batch b over query blocks qts; yields per
    (query-block, head) unit."""
    for qt in qts:
        for hl in range(H_LOC):
            bh = b * 2 + hl
            # o accumulators: qs 0,1 in o_ab[0], qs 2,3 in o_ab[1]
            o_ab = [P["ops"].tile([128, 2, 256], F32, name="o_ps", tag="o_ps")
                    for _ in range(2)]

            def o_reg(qs):
                return o_ab[qs // 2][:, qs % 2, 0:129]

            def emit_pv(kt, pt):
                jj = kt - 4 * qt
                for qs in range(max(jj, 0), 4):
                    if kt <= 4 * qt + qs:
                        # bank-wide start clear: only qs even starts its bank
                        nc.tensor.matmul(
                            o_reg(qs), pt[:, qs * 128:(qs + 1) * 128],
                            v_sb[:, bh, kt, 0:129],
                            start=(kt == 0 and qs % 2 == 0),
                            stop=(kt == 4 * qt + qs), skip_group_check=True)

            nkt = 4 * qt + 4
            prev = None
            for kt in range(nkt):
                jj = kt - 4 * qt
                q0 = 128 * jj if jj > 0 else 0
                st_ps = P["stps"].tile([128, 512], F32, name="st_ps", tag="st_ps")
                nc.tensor.matmul(st_ps[:, q0:512],
                                 qk_sb[:, 1, bh, kt * 128:(kt + 1) * 128],
                                 qk_sb[:, 0, bh, qt * 512 + q0:(qt + 1) * 512],
                                 start=True, stop=True)
                pt = P["ptp"].tile([128, 512], BF16, name="pt", tag="pt")
                nc.scalar.activation(out=pt[:, q0:512], in_=st_ps[:, q0:512],
                                     func=mybir.ActivationFunctionType.Exp,
                                     scale=float(SCALE))
                if jj >= 0:
                    # gpsimd (near-idle queue): diag causal mask post-exp
                    nc.gpsimd.tensor_mul(out=pt[:, q0:q0 + 128],
                                         in0=pt[:, q0:q0 + 128], in1=tri_sb[:])
                if prev is not None:
                    emit_pv(*prev)
                prev = (kt, pt)
            emit_pv(*prev)

            # batched o drain: 4 transposes into one PSUM bank, one copy out
            ot_ps = P["otps"].tile([128, 512], BF16, name="ot_ps",
                                   tag="ot_ps", padded_shape=[128, 1024])
            for qs in range(4):
                recip = P["osb"].tile([128, 1], F32, name="recip", tag="recip")
                nc.vector.reciprocal(out=recip[:],
                                     in_=o_ab[qs // 2][:, qs % 2, 128:129])
                o_t = P["osb"].tile([128, 128], BF16, name="o_t", tag="o_t")
                nc.vector.tensor_scalar_mul(out=o_t[:],
                                            in0=o_ab[qs // 2][:, qs % 2, 0:128],
                                            scalar1=recip[:])
                nc.tensor.matmul(ot_ps[:, qs * 128:(qs + 1) * 128], o_t[:],
                                 ident[:], is_transpose=True,
                                 start=(qs == 0), stop=(qs == 3),
                                 skip_group_check=True)
            tok0 = b * T + qt * 512
            nc.vector.tensor_copy(out=yT_sb[:, hl, tok0:tok0 + 512],
                                  in_=ot_ps[:])
            yield


def _emit_spill(nc, b, yT_sb, a2a_in):
    """Spill batch b's attention output to DRAM. Scalar queue: keeps the
    sync queue free for the x stream (the ~10us of DMA issues here would
    otherwise delay the next QKV block's x groups)."""
    for hl in range(H_LOC):
        for d in range(N_CORES):
            nc.scalar.dma_start(
                out=a2a_in[b][d, hl * 128:(hl + 1) * 128, :],
                in_=yT_sb[:, hl, b * T + 256 * d:b * T + 256 * (d + 1)])


def _emit_collective(nc, b, a2a_in, a2a_out):
    nc.gpsimd.collective_compute(
        "AllToAll", mybir.AluOpType.bypass,
        replica_groups=[list(range(N_CORES))],
        ins=[a2a_in[b].opt()], outs=[a2a_out[b].opt()],
    )


def _emit_readback(nc, b, a2a_out, ya_sb):
    # sync queue ONLY: it carries no compute, so the wait on the collective
    # cannot head-of-line block compute ops (the Tile scheduler interleaves
    # same-queue instructions, so a compute queue would stall).  Emitted
    # post-fence, so the single rearranged DMA has no concurrent writers.
    # ya[p, cc, col] = a2a_out[cc//2, (cc%2)*128 + p, col]
    nc.sync.dma_start(
        out=ya_sb[:, b, :, :],
        in_=a2a_out[b].rearrange("s (h p) c -> p (s h) c", h=2))


def _gen_proj(nc, hb, nfs, ya_sb, wproj_sb, y_d, P):
    """Output projection for batch hb over feature columns nfs; yields after
    each 512-feature column."""
    for nf in nfs:
        pj_ps = [P["pjps"].tile([128, 512], F32, name="pj_ps", tag="pj_ps")
                 for _ in range(2)]
        for cc in range(N_CCH):
            for mt in range(2):
                nc.tensor.matmul(pj_ps[mt][:],
                                 ya_sb[:, hb, cc, mt * 128:(mt + 1) * 128],
                                 wproj_sb[:, cc, nf * 512:(nf + 1) * 512],
                                 start=(cc == 0), stop=(cc == N_CCH - 1))
        for mt in range(2):
            o_sb = P["outp"].tile([128, 512], F32, name="o_sb", tag="o_sb")
            nc.vector.tensor_copy(out=o_sb[:], in_=pj_ps[mt][:])
            row0 = hb * 256 + mt * 128
            # scalar queue (idle in the proj phase): keeps y writes off the
            # sync queue, where they would sit behind the b1 readback's wait
            nc.scalar.dma_start(out=y_d[row0:row0 + 128,
                                        nf * 512:(nf + 1) * 512], in_=o_sb[:])
        yield


def _drain(gen):
    for _ in gen:
        pass


# ---------------- top-level emitter ----------------

def _emit(nc, tc, t_):
    x3_d, wqk_d, wv_d, wproj_d = t_["x3_d"], t_["wqk_d"], t_["wv_d"], t_["wproj_d"]
    cosT_d, sinTs_d, tri_d, y_d = t_["cosT_d"], t_["sinTs_d"], t_["tri_d"], t_["y_d"]
    a2a_in, a2a_out = t_["a2a_in"], t_["a2a_out"]

    ctx = contextlib.ExitStack()
    with ctx:
        pers = ctx.enter_context(tc.tile_pool(name="pers", bufs=1))
        ident = pers.tile([128, 128], BF16)
        tri_sb = pers.tile([128, 128], BF16)
        cmasks.make_identity(nc, ident[:])
        nc.gpsimd.dma_start(out=tri_sb[:], in_=tri_d)

        # wproj resident for the whole run; QKV-phase weights live in a
        # scoped pool freed before proj scratch opens.
        wp = ctx.enter_context(tc.tile_pool(name="wp", bufs=1))
        wproj_sb = wp.tile([128, N_CCH, C], BF16)

        qkv = ctx.enter_context(tc.tile_pool(name="qkv", bufs=1))
        qk_sb = qkv.tile([128, 2, 2 * H_LOC, T], BF16)  # (D, q/k, bh, T)
        v_sb = qkv.tile([128, 2 * H_LOC, T // 128, 132], BF16)
        yT_sb = qkv.tile([128, H_LOC, BT], BF16)
        nc.vector.memset(v_sb[:, :, :, 128:129], 1.0)

        yap = ctx.enter_context(tc.tile_pool(name="yap", bufs=1))
        ya_sb = yap.tile([128, 2, N_CCH, 256], BF16)

        # attention SBUF scratch (all phases)
        ptp = ctx.enter_context(tc.tile_pool(name="ptp", bufs=3))
        osb = ctx.enter_context(tc.tile_pool(name="osb", bufs=3))

        # attention PSUM pools for phases A+B (5 banks)
        catn = contextlib.ExitStack()
        AT1 = {
            "ptp": ptp, "osb": osb,
            "ops": catn.enter_context(
                tc.tile_pool(name="ops", bufs=2, space="PSUM")),
            "stps": catn.enter_context(
                tc.tile_pool(name="stps", bufs=2, space="PSUM")),
            "otps": catn.enter_context(
                tc.tile_pool(name="otps", bufs=1, space="PSUM")),
        }

        # QKV-phase SBUF (weights, x stream, rope scratch): freed before proj
        cq = contextlib.ExitStack()
        wp1 = cq.enter_context(tc.tile_pool(name="wp1", bufs=1))
        wqk_sb = wp1.tile([128, N_CCH, 512], BF16)
        wv_sb = wp1.tile([128, N_CCH, 256], BF16)
        cos_sb = wp1.tile([128, T], BF16)
        sin_sb = wp1.tile([128, T], BF16)
        for cc in range(N_CCH):
            nc.scalar.dma_start(out=wqk_sb[:, cc, :], in_=wqk_d[cc])
        nc.scalar.dma_start(out=wv_sb[:], in_=wv_d.transpose([1, 0, 2]))
        nc.gpsimd.dma_start(out=cos_sb[:], in_=cosT_d)
        nc.gpsimd.dma_start(out=sin_sb[:], in_=sinTs_d)
        xp = cq.enter_context(tc.tile_pool(name="xp", bufs=5))
        rp = cq.enter_context(tc.tile_pool(name="rp", bufs=4))

        def qkv_P(c):
            return {
                "xp": xp, "rp": rp,
                "qkps": c.enter_context(
                    tc.tile_pool(name="qkps", bufs=2, space="PSUM")),
                "vps": c.enter_context(
                    tc.tile_pool(name="vps", bufs=1, space="PSUM")),
            }

        att = dict(qk_sb=qk_sb, v_sb=v_sb, yT_sb=yT_sb, tri_sb=tri_sb,
                   ident=ident)

        # ---- phase A: QKV b0 (plus a warmup AllToAll so the b0 collective
        # doesn't pay first-use setup cost) ----
        nc.gpsimd.collective_compute(
            "AllToAll", mybir.AluOpType.bypass,
            replica_groups=[list(range(N_CORES))],
            ins=[t_["cc_warm_in"].opt()], outs=[t_["cc_warm_out"].opt()],
        )
        c1 = contextlib.ExitStack()
        with c1:
            _drain(_gen_qkv(nc, 0, x3_d, wqk_sb, wv_sb, cos_sb, sin_sb,
                            qk_sb, v_sb, qkv_P(c1)))

        # ---- phase B: attn b0 (8 units) 2:1 with QKV b1 (4 blocks);
        # b0 spill + AllToAll fire at the end of the phase.  wproj loads
        # here on gpsimd: off phase A's HBM window (only proj needs them)
        # and off the scalar/sync queues that carry v drains / x groups ----
        c2 = contextlib.ExitStack()
        with c2:
            for cc in range(N_CCH):
                nc.gpsimd.dma_start(out=wproj_sb[:, cc, :], in_=wproj_d[cc])
            gq1 = _gen_qkv(nc, 1, x3_d, wqk_sb, wv_sb, cos_sb, sin_sb,
                           qk_sb, v_sb, qkv_P(c2))
            ga0 = _gen_attention(nc, 0, (0, 1, 2, 3), P=AT1, **att)
            for u in "aaqaaqaaqaaq":
                next(gq1, None) if u == "q" else next(ga0, None)
            _drain(ga0)
            _drain(gq1)
            _emit_spill(nc, 0, yT_sb, a2a_in)
            _emit_collective(nc, 0, a2a_in, a2a_out)
        cq.close()   # free QKV weights + x stream SBUF
        catn.close() # free phase-A/B attention PSUM

        # ---- phase C: attn b1 (all 8 units; the b0 collective completes
        # in the background) ----
        c3 = contextlib.ExitStack()
        with c3:
            AT2 = {
                "ptp": ptp, "osb": osb,
                "ops": c3.enter_context(
                    tc.tile_pool(name="ops2", bufs=4, space="PSUM")),
                "stps": c3.enter_context(
                    tc.tile_pool(name="stps2", bufs=3, space="PSUM")),
                "otps": c3.enter_context(
                    tc.tile_pool(name="otps2", bufs=1, space="PSUM")),
            }
            _drain(_gen_attention(nc, 1, (0, 1, 2, 3), P=AT2, **att))
            _emit_spill(nc, 1, yT_sb, a2a_in)
            _emit_collective(nc, 1, a2a_in, a2a_out)

        # Scheduler-only fence: nothing from phase D may be hoisted above
        # phase C (the readbacks gate on collectives — hoisted above C's
        # attention or the x stream they stall the PE / sync queue).
        tc.no_sync_barrier()
        _emit_readback(nc, 0, a2a_out, ya_sb)
        _emit_readback(nc, 1, a2a_out, ya_sb)

        # ---- phase D: proj hb0 (hides the b1 AllToAll), then proj hb1 ----
        c4 = contextlib.ExitStack()
        with c4:
            PJ = {"outp": c4.enter_context(tc.tile_pool(name="outp2", bufs=3)),
                  "pjps": c4.enter_context(
                      tc.tile_pool(name="pjps2", bufs=4, space="PSUM"))}
            _drain(_gen_proj(nc, 0, (0, 1, 2, 3), ya_sb, wproj_sb, y_d, PJ))
            _drain(_gen_proj(nc, 1, (0, 1, 2, 3), ya_sb, wproj_sb, y_d, PJ))


# ---------------- host side ----------------

def _rope_tables():
    inv_freq = 1.0 / (ROPE_BASE ** (np.arange(0, D, 2, dtype=np.float32) / D))
    ang = np.arange(T, dtype=np.float32)[:, None] * inv_freq[None, :]   # (T, D/2)
    cos = np.concatenate([np.cos(ang), np.cos(ang)], axis=-1).astype(np.float32)
    sin = np.concatenate([np.sin(ang), np.sin(ang)], axis=-1).astype(np.float32)
    cosT = np.ascontiguousarray(cos.T)                                  # (D, T)
    sinTs = np.ascontiguousarray(sin.T)
    sinTs[0:64, :] *= -1.0
    return (cosT.astype(ml_dtypes.bfloat16), sinTs.astype(ml_dtypes.bfloat16))


def _tri_mask():
    kp = np.arange(128)[:, None]
    qf = np.arange(128)[None, :]
    return (kp <= qf).astype(ml_dtypes.bfloat16)                        # (128, 128)


def prep_in_maps(x, w_qkv, w_proj):
    bf = ml_dtypes.bfloat16
    x = np.asarray(x, dtype=np.float32)
    w_qkv = np.asarray(w_qkv, dtype=np.float32)
    w_proj = np.asarray(w_proj, dtype=np.float32)

    # (N_XB, C, 512): token-block-major transposed x
    x3 = np.ascontiguousarray(
        x.reshape(N_XB, 512, C).transpose(0, 2, 1)).astype(bf)
    wprojT = np.ascontiguousarray(w_proj.T).reshape(N_CCH, 128, C).astype(bf)
    cosT, sinTs = _rope_tables()
    tri = _tri_mask()

    in_maps = []
    for r in range(N_CORES):
        rows = slice(256 * r, 256 * (r + 1))
        wq = np.ascontiguousarray(w_qkv[0 * C:1 * C][rows].T).reshape(N_CCH, 128, 256)
        wk = np.ascontiguousarray(w_qkv[1 * C:2 * C][rows].T).reshape(N_CCH, 128, 256)
        wv = np.ascontiguousarray(w_qkv[2 * C:3 * C][rows].T).reshape(N_CCH, 128, 256)
        wqk = np.concatenate([wq, wk], axis=2)                           # (16,128,512)
        in_maps.append({
            "x3": x3, "wqk": np.ascontiguousarray(wqk).astype(bf),
            "wv": wv.astype(bf), "wproj": wprojT, "cosT": cosT,
            "sinTs": sinTs, "tri": tri,
        })
    return in_maps


def assemble(results):
    y0 = np.concatenate([results[r]["y"][0:256] for r in range(N_CORES)], axis=0)
    y1 = np.concatenate([results[r]["y"][256:512] for r in range(N_CORES)], axis=0)
    return np.stack([y0, y1], axis=0).reshape(B, T, C).astype(np.float32)


_CACHED_NC = None


def kernel(x, w_qkv, w_proj):
    global _CACHED_NC
    if _CACHED_NC is None:
        _CACHED_NC = build()
    in_maps = prep_in_maps(x, w_qkv, w_proj)
    res = run_bass_kernel_spmd(_CACHED_NC, in_maps, list(range(N_CORES)))
    return assemble(res.results)
